# revision 28
# baseline (speedup 1.0000x reference)
"""Trainium2 Bass kernel for nn_AutoIFS_89378269430151 (moe_routing).

Data-parallel over batch across 8 NeuronCores; all params replicated and
RESIDENT in SBUF (loaded once). Feature-major layout on chip (features on
partitions, batch on free dim).

Embedding gather: SWDGE dma_gather over a (25000, 256) fp8-e3m4 (x16
pre-scaled) reshaped view of the table with int16 indices (idx = x>>2);
the quarter (x&3) is selected on DVE with predicated copies over a uint16
view (byte pairs share the per-sample quarter mask, halving DVE elements;
quarter-0 baseline copied on the scalar engine), then PE-transposed in
fp8 (16-bit lanes: the transpose output has element step 2, consumers
read the even-byte plane) into per-phase feature-major tiles. Each batch
tile uses 16 sub-gathers of 1024 indices issued phase-major over the 4
SWDGE queues (idx DMA chunked per round, issue-order layout; gather-count
register materialized once) so each ~9us descriptor-generation round
completes one 4-k-block "phase" across all four 128-sample column blocks.

Compute: share tower in bf16 at the 215ns/MM roofline; gate (hypernet)
tower in fp8-e4m3 DoubleRow (2x K per MM). Tile 0's share-l0 first half is
K-phase-split: it starts consuming gather phases ~20us into the run
(accumulating PSUM across phases) instead of waiting for the full tile-0
select, with sW0 loaded in four 1MB K-group chunks so weight DMA pipelines
with the gather rounds. The x16 table pre-scale is undone in the l0
activation epilogues. The domain/task/output "tail" of tile t is emitted as
a generator interleaved into tile t+1's towers; the final output epilogue
is emitted stage-major across the two tasks so PE/ACT/DVE pipeline at the
end of the run. Output is written untransposed (2, 128, B/8) in bf16 and
transposed/upcast on the host.
"""

import os

os.environ.setdefault("JAX_PLATFORMS", "axon")

import numpy as np

# ---- problem constants (hardcoded; must match reference.py) ----
B, F, L = 16384, 32, 64
FEAT, DOM, R = 100000, 8, 16
D_IN = F * L  # 2048
NCORES = 8
BC = B // NCORES  # 2048 samples per core
NB = 512  # batch tile (free dim)
NT = BC // NB  # 4 batch tiles per core
P = 128

V4 = FEAT // 4  # 25000 rows in reshaped table
E4 = 4 * L  # 256 elements per reshaped row (fp8 -> 256B)
NSUB = 16  # sub-gathers per batch tile (4 phases x 4 col-blocks)
NIDX = NB * F // NSUB  # 1024 indices per sub-gather
SLOT = NIDX // P  # 8 slots per sub-gather
NPH = 4  # k-phases per tile (4 k-blocks each)
ESCALE = 16.0  # table pre-scale (undone in l0 activation)

_BUILT = {}


def build():
    if "nc" in _BUILT:
        return _BUILT["nc"]
    import concourse.bass as bass
    from concourse import bacc
    import concourse.mybir as mybir
    import concourse.tile as tile

    dt = mybir.dt
    AF = mybir.ActivationFunctionType
    OP = mybir.AluOpType
    f32, i16, bf16 = dt.float32, dt.int16, dt.bfloat16
    f8 = dt.float8e3
    f8e4 = dt.float8e4
    u16 = dt.uint16
    MPM = mybir.MatmulPerfMode

    nc = bacc.Bacc(None, target_bir_lowering=False, num_swdge_queues=4)

    def din(name, shape, dtype=f32):
        return nc.dram_tensor(name, shape, dtype, kind="ExternalInput")

    # ---- DRAM inputs (per core) ----
    emb4 = din("emb4", [V4, E4], f8)  # reshaped fp8 embedding table (x16)
    idxd = din("idxd", [P, NT * NSUB * NIDX // 16], i16)  # wrapped gather idx
    mqd = din("mqd", [P, NT * 3 * P], dt.uint8)  # quarter masks q=1,2,3
    dmb = din("dmb", [P, NT * NB], bf16)  # (d[col]==p//16) mask
    dm8 = din("dm8", [8, NT * NB], bf16)  # (d[col]==r) mask

    sW0k = [din(f"sW0k{g}", [512, 1024], bf16) for g in range(NPH)]
    gW0 = din("gW0", [D_IN, 1024], dt.float8e4)
    sW1 = din("sW1", [1024, 512], bf16)
    gW1 = din("gW1", [1024, 512], dt.float8e4)
    sb0 = din("sb0", [1024, 1])
    gb0 = din("gb0", [1024, 1])
    sb1 = din("sb1", [512, 1])
    gb1 = din("gb1", [512, 1])

    Dk0 = din("Dk0", [512, 256], bf16)
    Db0 = din("Db0", [256, 1])
    DA0p = din("DA0p", [512, 128], bf16)  # [i, dom*16+r]
    DB0p = din("DB0p", [128, 256], bf16)  # [dom*16+r, o]
    Dlb0p = din("Dlb0p", [8, 256], bf16)  # [dom, o]

    Tk0 = din("Tk0", [256, 128], bf16)
    TA0p = din("TA0p", [256, 32], bf16)  # [TA0[0] | TA0[1]]
    TB0p_0 = din("TB0p_0", [32, 128], bf16)  # [TB0[0]; 0]
    TB0p_1 = din("TB0p_1", [32, 128], bf16)  # [0; TB0[1]]
    bias_d0 = din("bias_d0", [128, 1])  # Tb0 + Tlb0[0]
    bias_d1 = din("bias_d1", [128, 1])  # Tb0 + Tlb0[1]
    bias_s = din("bias_s", [128, 1])  # Tb0
    bias_l0 = din("bias_l0", [128, 1])  # Tlb0[0]
    bias_l1 = din("bias_l1", [128, 1])  # Tlb0[1]

    pk1_0 = din("pk1_0", [128, 32], bf16)  # cols 0:16 TA1[0], col 16 Tk1
    pk1_1 = din("pk1_1", [128, 32], bf16)
    TA1_0 = din("TA1_0", [128, 16], bf16)
    TA1_1 = din("TA1_1", [128, 16], bf16)
    # rank-1 row-broadcast matrices for the i=1 epilogue
    FD1_0 = din("FD1_0", [32, 128], bf16)  # rows 0:16 = TB1[0] col, row 16 = 1
    FD1_1 = din("FD1_1", [32, 128], bf16)
    LB1_0 = din("LB1_0", [16, 128], bf16)  # = TB1[0] broadcast along cols
    LB1_1 = din("LB1_1", [16, 128], bf16)
    K1b = din("K1b", [128, 128], bf16)  # = Tk1 broadcast along cols
    # per-partition scalar constants (value replicated on 128 partitions)
    c_d0 = din("c_d0", [128, 1])  # Tb1 + Tlb1[0]
    c_d1 = din("c_d1", [128, 1])  # Tb1 + Tlb1[1]
    c_s = din("c_s", [128, 1])  # Tb1
    c_l0 = din("c_l0", [128, 1])  # Tlb1[0]
    c_l1 = din("c_l1", [128, 1])  # Tlb1[1]

    HWp = din("HWp", [512, 8], bf16)  # [h0W | h1W]
    hb = din("hb", [8, 1])

    ident8 = din("ident8", [128, 128], f8)  # fp8e3 identity for transposes
    G8sel = din("G8sel", [8, 8 * 128], bf16)  # row-broadcast for gates rows

    out = nc.dram_tensor("out", [2, 128, BC], bf16, kind="ExternalOutput")

    with tile.TileContext(nc) as tc:
        with (
            tc.tile_pool(name="const", bufs=1) as cp,
            tc.tile_pool(name="gpool", bufs=12) as gp,
            tc.tile_pool(name="selpool", bufs=5) as sp,
            tc.tile_pool(name="xt", bufs=8) as xp,
            tc.tile_pool(name="xtf", bufs=8) as xfp,
            tc.tile_pool(name="hpool", bufs=1) as hp,
            tc.tile_pool(name="h1pool", bufs=2) as h1p,
            tc.tile_pool(name="gatep", bufs=2) as gtp,
            tc.tile_pool(name="task", bufs=1) as tkp,
            tc.tile_pool(name="small", bufs=2) as smp,
            tc.tile_pool(name="scratch", bufs=10) as scr,
            tc.tile_pool(name="scratchf", bufs=2) as scrf,
            tc.tile_pool(name="bounce", bufs=3) as bnc,
            tc.tile_pool(name="ps", bufs=4, space="PSUM") as psp,
            tc.tile_pool(name="tail", bufs=2, space="PSUM") as tlp,
            tc.tile_pool(name="pt", bufs=2, space="PSUM") as ptp,
        ):
            def pst_():
                return psp.tile([128, 512], dt.float32, space="PSUM", tag="ps",
                                name="pstile")

            def tlt_():
                return tlp.tile([128, 512], dt.float32, space="PSUM", tag="tl",
                                name="tltile")

            # ---- resident tensors ----
            def load_const(t, shape, dtype=f32):
                s = cp.tile(shape, dtype, tag=t.name)
                nc.sync.dma_start(out=s[:], in_=t[:])
                return s

            ICOLS = NSUB * NIDX // 16  # idx cols per tile (1024)
            RCOLS = ICOLS // NPH  # idx cols per descgen round (256)

            def load_tile_inputs(t, sync_queue=False):
                eng = nc.sync if sync_queue else nc.scalar
                # idx memory is laid out in ISSUE order (u = p*4+cc), so one
                # chunk per descgen round: round p's gathers only wait for
                # chunk p's (much earlier) completion
                idx_t = []
                for p in range(NPH):
                    c = smp.tile([P, RCOLS], i16, tag=f"idx{p}", name="idx_t")
                    eng.dma_start(
                        out=c[:],
                        in_=idxd[:, t * ICOLS + p * RCOLS
                                 : t * ICOLS + (p + 1) * RCOLS],
                    )
                    idx_t.append(c)
                mq_t = smp.tile([P, 3, P], dt.uint8, tag="mq", name="mq_t")
                eng.dma_start(
                    out=mq_t[:], in_=mqd[:, t * 3 * P : (t + 1) * 3 * P]
                )
                mbc_t = smp.tile([128, NB], bf16, tag="dmb", name="mbc_t")
                nc.scalar.dma_start(
                    out=mbc_t[:], in_=dmb[:, t * NB : (t + 1) * NB]
                )
                m8_t = smp.tile([8, NB], bf16, tag="dm8", name="m8_t")
                nc.scalar.dma_start(
                    out=m8_t[:], in_=dm8[:, t * NB : (t + 1) * NB]
                )
                return idx_t, mq_t, (mbc_t, m8_t)

            # materialize the gather-count register once: a per-gather
            # MOVE costs ~400ns of GpSimd dispatch, serializing in front of
            # the first descgen at startup
            nidx_reg = nc.gpsimd.to_reg(NIDX)

            def issue_gathers(idx_t, t):
                """Issue all 16 sub-gathers of tile t, phase-major: round p
                puts one sub-gather on each of the 4 queues (queue = column
                block cc), so each descgen round completes one k-phase."""
                gs = []
                for p in range(NPH):
                    for cc in range(4):
                        g = gp.tile([P, SLOT, E4], f8, tag="g", name="g")
                        c0 = cc * (NIDX // 16)
                        nc.gpsimd.dma_gather(
                            out_ap=g[:],
                            in_ap=emb4[:],
                            idxs_ap=idx_t[p][:, c0 : c0 + NIDX // 16],
                            num_idxs=NIDX,
                            num_idxs_reg=nidx_reg,
                            elem_size=E4,
                            single_packet=False,
                            queue_num=cc,
                        )
                        gs.append(g)
                return gs

            # tile-0 inputs + gathers FIRST so the idx DMA and SWDGE
            # descriptor generation are not stuck behind the weight loads
            idx_c, mq_c, masks_c = load_tile_inputs(0, sync_queue=True)
            id8_s = load_const(ident8, [128, 128], f8)
            gs_cur = issue_gathers(idx_c, 0)

            # sW0 k-groups next: tile-0's phased share-l0 consumes them in
            # order, pipelined with the gather rounds
            sW0k_s = [load_const(sW0k[g], [128, 4, 1024], bf16)
                      for g in range(NPH)]
            sb0_s = load_const(sb0, [128, 8, 1])
            sb1_s = load_const(sb1, [128, 4, 1])
            sW1_s = load_const(sW1, [128, 8, 512], bf16)
            gW0_s = load_const(gW0, [128, 16, 1024], dt.float8e4)
            gW1_s = load_const(gW1, [128, 8, 512], dt.float8e4)
            gb0_s = load_const(gb0, [128, 8, 1])
            gb1_s = load_const(gb1, [128, 4, 1])
            Dk0_s = load_const(Dk0, [128, 4, 256], bf16)
            Db0_s = load_const(Db0, [128, 2, 1])
            DA0p_s = load_const(DA0p, [128, 4, 128], bf16)
            DB0p_s = load_const(DB0p, [128, 256], bf16)
            Dlb0p_s = load_const(Dlb0p, [8, 256], bf16)
            Tk0_s = load_const(Tk0, [128, 2, 128], bf16)
            TA0p_s = load_const(TA0p, [128, 2, 32], bf16)
            TB0p_0s = load_const(TB0p_0, [32, 128], bf16)
            TB0p_1s = load_const(TB0p_1, [32, 128], bf16)
            bd0_s = load_const(bias_d0, [128, 1])
            bd1_s = load_const(bias_d1, [128, 1])
            bs_s = load_const(bias_s, [128, 1])
            bl0_s = load_const(bias_l0, [128, 1])
            bl1_s = load_const(bias_l1, [128, 1])
            pk1_0s = load_const(pk1_0, [128, 32], bf16)
            pk1_1s = load_const(pk1_1, [128, 32], bf16)
            TA1_0s = load_const(TA1_0, [128, 16], bf16)
            TA1_1s = load_const(TA1_1, [128, 16], bf16)
            FD1_s = [load_const(FD1_0, [32, 128], bf16),
                     load_const(FD1_1, [32, 128], bf16)]
            LB1_s = [load_const(LB1_0, [16, 128], bf16),
                     load_const(LB1_1, [16, 128], bf16)]
            K1b_s = load_const(K1b, [128, 128], bf16)
            cd_s = [load_const(c_d0, [128, 1]), load_const(c_d1, [128, 1])]
            cs_s = load_const(c_s, [128, 1])
            cl_s = [load_const(c_l0, [128, 1]), load_const(c_l1, [128, 1])]
            HW_s = load_const(HWp, [128, 4, 8], bf16)
            hb_s = load_const(hb, [8, 1])
            G8_s = load_const(G8sel, [8, 8, 128], bf16)

            def mk_phase_tiles():
                xT = [xp.tile([128, 4, 512], bf16, tag="xT", name="xTp")
                      for _ in range(NPH)]
                xTf = [xfp.tile([128, 4, 512], f8e4, tag="xTf", name="xTfp")
                       for _ in range(NPH)]
                return xT, xTf

            def make_selector(gs, mq_t, xT_phs, xTf_phs):
                state = {"jd": 0, "jp": 0, "sels": []}

                def sel_dve(n=1):
                    for _ in range(n):
                        u = state["jd"]
                        if u >= NSUB:
                            return
                        state["jd"] += 1
                        s4 = (u % 4) * NPH + u // 4
                        g = gs[u]
                        gu = g[:].bitcast(u16)  # [128, SLOT, 128]
                        sel = sp.tile([P, SLOT, L], f8, tag="sel", name="sel")
                        nc.scalar.activation(out=sel[:], in_=g[:, :, 0:L],
                                             func=AF.Identity)
                        selu = sel[:].bitcast(u16)  # [128, SLOT, 32]
                        for q in range(1, 4):
                            nc.vector.copy_predicated(
                                out=selu[:],
                                mask=mq_t[:, q - 1, s4 * SLOT : (s4 + 1) * SLOT,
                                          None].to_broadcast([P, SLOT, 32]),
                                data=gu[:, :, q * 32 : (q + 1) * 32],
                            )
                        state["sels"].append(sel)

                def sel_pe(n=1):
                    for _ in range(n):
                        u = state["jp"]
                        if u >= NSUB or u >= state["jd"]:
                            return
                        state["jp"] += 1
                        p, cc = u // 4, u % 4
                        sel = state["sels"][u]
                        # fp8 PE transpose writes 16-bit lanes: the output
                        # element step must be 2, so write/read the even
                        # byte plane of a [.., 128, 2] tile
                        pt = ptp.tile([128, 4, 128, 2], f8, space="PSUM",
                                      tag="pt", name="pt")
                        for f2 in range(4):
                            nc.tensor.transpose(
                                out=pt[:, f2, :, 0],
                                in_=sel[:, 2 * f2 : 2 * f2 + 2, :],
                                identity=id8_s[:],
                            )
                        nc.scalar.activation(
                            out=xT_phs[p][:, :, cc * 128 : (cc + 1) * 128],
                            in_=pt[:, :, :, 0], func=AF.Identity,
                        )
                        nc.vector.tensor_copy(
                            out=xTf_phs[p][:, :, cc * 128 : (cc + 1) * 128],
                            in_=pt[:, :, :, 0],
                        )

                return sel_dve, sel_pe

            def tower_l0_dr(Wf_s, bias_tile, xTf_phs, aux):
                h0 = hp.tile([128, 8, 512], f8e4, tag="h0", name="h0f")
                for half in range(2):
                    pst = [pst_() for _ in range(4)]
                    for kk in range(8):
                        ph, jj = kk // 2, kk % 2
                        for m in range(4):
                            nc.tensor.matmul(
                                out=pst[m][:],
                                lhsT=Wf_s[:, 2 * kk : 2 * kk + 2,
                                          half * 512 + m * 128
                                          : half * 512 + (m + 1) * 128],
                                rhs=xTf_phs[ph][:, 2 * jj : 2 * jj + 2, :],
                                start=(kk == 0),
                                stop=(kk == 7),
                                perf_mode=MPM.DoubleRow,
                            )
                        if kk < 6:
                            aux()
                    for m in range(4):
                        mm = half * 4 + m
                        nc.scalar.activation(
                            out=h0[:, mm, :], in_=pst[m][:], func=AF.Relu,
                            bias=bias_tile[:, mm, :], scale=1.0 / ESCALE,
                        )
                    aux()
                    aux()
                return h0

            def tower_l1_dr(h0f, W1f_s, b1s, aux):
                h1 = h1p.tile([128, 4, 512], bf16, tag="h1", name="h1")
                pst = [pst_() for _ in range(4)]
                for kk in range(4):
                    for m in range(4):
                        nc.tensor.matmul(
                            out=pst[m][:],
                            lhsT=W1f_s[:, 2 * kk : 2 * kk + 2,
                                       m * 128 : (m + 1) * 128],
                            rhs=h0f[:, 2 * kk : 2 * kk + 2, :],
                            start=(kk == 0),
                            stop=(kk == 3),
                            perf_mode=MPM.DoubleRow,
                        )
                    if kk < 3:
                        aux()
                for m in range(4):
                    nc.scalar.activation(
                        out=h1[:, m, :], in_=pst[m][:], func=AF.Relu,
                        bias=b1s[:, m, :],
                    )
                aux()
                return h1

            def tower_l0(Wk_s, bias_tile, xT_phs, aux, scale=1.0,
                         phase_cb=None):
                h0 = hp.tile([128, 8, 512], bf16, tag="h0", name="h0")
                for half in range(2):
                    pst = [pst_() for _ in range(4)]
                    for k in range(16):
                        p, kk = k // 4, k % 4
                        if phase_cb is not None and half == 0 and kk == 0:
                            phase_cb(p)
                        for m in range(4):
                            nc.tensor.matmul(
                                out=pst[m][:],
                                lhsT=Wk_s[p][:, kk,
                                             half * 512 + m * 128
                                             : half * 512 + (m + 1) * 128],
                                rhs=xT_phs[p][:, kk, :],
                                start=(k == 0),
                                stop=(k == 15),
                            )
                        if k < 12:
                            aux()
                    # epilogues BEFORE the trailing aux calls so the next
                    # half's accumulator WAR clears without queueing behind
                    # select copies on ACT
                    for m in range(4):
                        mm = half * 4 + m
                        nc.scalar.activation(
                            out=h0[:, mm, :], in_=pst[m][:], func=AF.Relu,
                            bias=bias_tile[:, mm, :], scale=scale,
                        )
                    for _ in range(4):
                        aux()
                return h0

            def tower_l1(h0, W1_s, b1s, aux):
                h1 = h1p.tile([128, 4, 512], bf16, tag="h1", name="h1")
                pst = [pst_() for _ in range(4)]
                for k in range(8):
                    for m in range(4):
                        nc.tensor.matmul(
                            out=pst[m][:],
                            lhsT=W1_s[:, k, m * 128 : (m + 1) * 128],
                            rhs=h0[:, k, :],
                            start=(k == 0),
                            stop=(k == 7),
                        )
                    if k < 6:
                        aux()
                for m in range(4):
                    nc.scalar.activation(
                        out=h1[:, m, :], in_=pst[m][:], func=AF.Relu,
                        bias=b1s[:, m, :],
                    )
                aux()
                aux()
                return h1

            noop = lambda n=1: None

            def epi_relu(pstile, bias, tag):
                o = tkp.tile([128, 512], bf16, tag=tag, name="epi")
                nc.scalar.activation(out=o[:], in_=pstile[:], func=AF.Relu,
                                     bias=bias[:])
                return o

            def epi_add(pstile, bias, tag):
                o = tkp.tile([128, 512], bf16, tag=tag, name="epi")
                nc.scalar.activation(out=o[:], in_=pstile[:],
                                     func=AF.Identity, bias=bias[:])
                return o

            def tail_gen(t, h1s, gates_get, masks_t, last=False):
                """Domain + task layers + output for tile t, as a generator
                whose chunks are interleaved into the next tile's towers."""
                mbc, mask8 = masks_t
                psR = tlt_()
                for k in range(4):
                    nc.tensor.matmul(out=psR[:], lhsT=DA0p_s[:, k, :],
                                     rhs=h1s[:, k, :],
                                     start=(k == 0), stop=(k == 3))
                yield
                rmask = tkp.tile([128, 512], bf16, tag="rmask", name="rmask")
                nc.vector.tensor_tensor(out=rmask[:], in0=psR[:], in1=mbc[:],
                                        op=OP.mult)
                yield
                share = tkp.tile([128, 2, 512], bf16, tag="share", name="share")
                lora = tkp.tile([128, 2, 512], bf16, tag="lora", name="lora")
                ddnn = tkp.tile([128, 2, 512], bf16, tag="ddnn", name="ddnn")
                for m in range(2):
                    pss = tlt_()
                    for k in range(4):
                        nc.tensor.matmul(
                            out=pss[:],
                            lhsT=Dk0_s[:, k, m * 128 : (m + 1) * 128],
                            rhs=h1s[:, k, :],
                            start=(k == 0), stop=(k == 3),
                        )
                    yield
                    psl = tlt_()
                    nc.tensor.matmul(out=psl[:],
                                     lhsT=DB0p_s[:, m * 128 : (m + 1) * 128],
                                     rhs=rmask[:], start=True, stop=False)
                    nc.tensor.matmul(out=psl[:],
                                     lhsT=Dlb0p_s[:, m * 128 : (m + 1) * 128],
                                     rhs=mask8[:], start=False, stop=True)
                    nc.scalar.activation(out=share[:, m, :], in_=pss[:],
                                         func=AF.Identity, bias=Db0_s[:, m, :])
                    nc.vector.tensor_copy(out=lora[:, m, :], in_=psl[:])
                    yield
                    tmp = scrf.tile([128, 512], f32, tag="owf", name="tmp")
                    nc.vector.tensor_tensor(out=tmp[:], in0=share[:, m, :],
                                            in1=lora[:, m, :], op=OP.add)
                    nc.scalar.activation(out=ddnn[:, m, :], in_=tmp[:],
                                         func=AF.Relu)
                    yield

                # ---- task layer i=0 ----
                def aprod2(rhs3):
                    pa = tlt_()
                    nc.tensor.matmul(out=pa[0:32, :], lhsT=TA0p_s[:, 0, :],
                                     rhs=rhs3[:, 0, :], start=True, stop=False)
                    nc.tensor.matmul(out=pa[0:32, :], lhsT=TA0p_s[:, 1, :],
                                     rhs=rhs3[:, 1, :], start=False, stop=True)
                    ab = bnc.tile([32, 512], bf16, tag="Abuf", name="ab")
                    nc.vector.tensor_copy(out=ab[:], in_=pa[0:32, :])
                    return ab

                def mm_k2(pstile, lhs_tile, rhs3, last):
                    nc.tensor.matmul(out=pstile[:], lhsT=lhs_tile[:, 0, :],
                                     rhs=rhs3[:, 0, :], start=True, stop=False)
                    nc.tensor.matmul(out=pstile[:], lhsT=lhs_tile[:, 1, :],
                                     rhs=rhs3[:, 1, :], start=False, stop=last)

                ab_d = aprod2(ddnn)
                yield
                ps_d0 = tlt_()
                mm_k2(ps_d0, Tk0_s, ddnn, last=False)
                nc.tensor.matmul(out=ps_d0[:], lhsT=TB0p_0s[:], rhs=ab_d[:],
                                 start=False, stop=True)
                t_dnn0 = epi_relu(ps_d0, bd0_s, "tdnn0")
                yield
                ps_d1 = tlt_()
                mm_k2(ps_d1, Tk0_s, ddnn, last=False)
                nc.tensor.matmul(out=ps_d1[:], lhsT=TB0p_1s[:], rhs=ab_d[:],
                                 start=False, stop=True)
                t_dnn1 = epi_add(ps_d1, bd1_s, "tdnn1")
                yield
                ab_s = aprod2(share)
                yield
                ps_sl0 = tlt_()
                nc.tensor.matmul(out=ps_sl0[:], lhsT=TB0p_0s[:], rhs=ab_s[:],
                                 start=True, stop=True)
                t_sl0 = epi_relu(ps_sl0, bl0_s, "tsl0")
                yield
                ps_sl1 = tlt_()
                nc.tensor.matmul(out=ps_sl1[:], lhsT=TB0p_1s[:], rhs=ab_s[:],
                                 start=True, stop=True)
                t_sl1 = epi_add(ps_sl1, bl1_s, "tsl1")
                yield
                ab_l = aprod2(lora)
                yield
                ps_ll0 = tlt_()
                nc.tensor.matmul(out=ps_ll0[:], lhsT=TB0p_0s[:], rhs=ab_l[:],
                                 start=True, stop=True)
                t_lo0 = epi_relu(ps_ll0, bl0_s, "tlo0")
                yield
                ps_ll1 = tlt_()
                nc.tensor.matmul(out=ps_ll1[:], lhsT=TB0p_1s[:], rhs=ab_l[:],
                                 start=True, stop=True)
                t_lo1 = epi_add(ps_ll1, bl1_s, "tlo1")
                yield
                ps_S = tlt_()
                mm_k2(ps_S, Tk0_s, share, last=True)
                t_sh0 = epi_relu(ps_S, bs_s, "tsh0")
                t_sh1 = epi_add(ps_S, bs_s, "tsh1")
                yield
                ps_L = tlt_()
                mm_k2(ps_L, Tk0_s, lora, last=True)
                t_ls0 = epi_relu(ps_L, bs_s, "tls0")
                t_ls1 = epi_add(ps_L, bs_s, "tls1")
                yield

                # ---- task layer i=1 + output, stage-major across tasks ----
                # everything that does not need the gates runs BEFORE
                # gates_get(), so the gate-dependent chain exposed at the
                # end of the run (inside the HW throttle window) is minimal
                pk1s = [pk1_0s, pk1_1s]
                TA1s = [TA1_0s, TA1_1s]
                tdnns = [t_dnn0, t_dnn1]
                tshs = [t_sh0, t_sh1]
                tlos = [t_lo0, t_lo1]
                slts = [t_sl0, t_sl1]
                lsts = [t_ls0, t_ls1]

                fbs, lbs = [], []
                for tt in range(2):
                    pf = tlt_()
                    nc.tensor.matmul(out=pf[0:32, :], lhsT=pk1s[tt][:],
                                     rhs=tdnns[tt][:], start=True, stop=True)
                    fb = bnc.tile([32, 512], bf16, tag="Fbuf", name="fb")
                    nc.vector.tensor_copy(out=fb[:], in_=pf[0:32, :])
                    fbs.append(fb)
                    yield
                for tt in range(2):
                    pl = tlt_()
                    nc.tensor.matmul(out=pl[0:16, :], lhsT=TA1s[tt][:],
                                     rhs=tlos[tt][:], start=True, stop=True)
                    lb = bnc.tile([16, 512], bf16, tag="Abuf", name="lb")
                    nc.vector.tensor_copy(out=lb[:], in_=pl[0:16, :])
                    lbs.append(lb)
                    yield
                a1s = []
                for tt in range(2):
                    b_ss = tlt_()
                    nc.tensor.matmul(out=b_ss[:], lhsT=K1b_s[:],
                                     rhs=tshs[tt][:], start=True, stop=True)
                    a1t = scr.tile([128, 512], bf16, tag="ow", name="a1t")
                    nc.scalar.activation(out=a1t[:], in_=b_ss[:],
                                         func=AF.Identity, bias=cs_s[:])
                    a1s.append(a1t)
                    yield
                b1s = []
                for tt in range(2):
                    b_ll = tlt_()
                    nc.tensor.matmul(out=b_ll[:], lhsT=LB1_s[tt][:],
                                     rhs=lbs[tt][:], start=True, stop=True)
                    b1t = scr.tile([128, 512], bf16, tag="ow", name="b1t")
                    nc.scalar.activation(out=b1t[:], in_=b_ll[:],
                                         func=AF.Identity, bias=cl_s[tt][:])
                    b1s.append(b1t)
                    yield

                gates = gates_get()
                a2s = []
                for tt in range(2):
                    b_g0 = tlt_()
                    nc.tensor.matmul(out=b_g0[:], lhsT=G8_s[:, 4 * tt, :],
                                     rhs=gates[:], start=True, stop=True)
                    a2t = scr.tile([128, 512], bf16, tag="ow", name="a2t")
                    nc.vector.tensor_tensor(out=a2t[:], in0=a1s[tt][:],
                                            in1=b_g0[:], op=OP.mult)
                    a2s.append(a2t)
                    yield
                b2s = []
                for tt in range(2):
                    b_g1 = tlt_()
                    nc.tensor.matmul(out=b_g1[:], lhsT=G8_s[:, 4 * tt + 1, :],
                                     rhs=gates[:], start=True, stop=True)
                    b2t = scr.tile([128, 512], bf16, tag="ow", name="b2t")
                    nc.vector.tensor_tensor(out=b2t[:], in0=b1s[tt][:],
                                            in1=b_g1[:], op=OP.mult)
                    b2s.append(b2t)
                    yield
                # pre-sum the gate terms (tree) so only two serial vector
                # ops remain after the final w1t activation
                q1s = []
                for tt in range(2):
                    q1 = scr.tile([128, 512], bf16, tag="ow", name="q1t")
                    nc.vector.tensor_tensor(out=q1[:], in0=a2s[tt][:],
                                            in1=b2s[tt][:], op=OP.add)
                    q1s.append(q1)
                yield
                m1s = []
                for tt in range(2):
                    b_g2 = tlt_()
                    nc.tensor.matmul(out=b_g2[:], lhsT=G8_s[:, 4 * tt + 2, :],
                                     rhs=gates[:], start=True, stop=True)
                    m1t = scr.tile([128, 512], bf16, tag="ow", name="m1t")
                    nc.vector.tensor_tensor(out=m1t[:], in0=b_g2[:],
                                            in1=slts[tt][:], op=OP.mult)
                    m1s.append(m1t)
                    yield
                q2s = []
                for tt in range(2):
                    b_g3 = tlt_()
                    nc.tensor.matmul(out=b_g3[:], lhsT=G8_s[:, 4 * tt + 3, :],
                                     rhs=gates[:], start=True, stop=True)
                    m3t = scr.tile([128, 512], bf16, tag="ow", name="m3t")
                    nc.vector.tensor_tensor(out=m3t[:], in0=b_g3[:],
                                            in1=lsts[tt][:], op=OP.mult)
                    q2 = scr.tile([128, 512], bf16, tag="ow", name="q2t")
                    nc.vector.tensor_tensor(out=q2[:], in0=m1s[tt][:],
                                            in1=m3t[:], op=OP.add)
                    q2s.append(q2)
                    yield
                for tt in range(2):
                    b_w = tlt_()
                    nc.tensor.matmul(out=b_w[:], lhsT=FD1_s[tt][:],
                                     rhs=fbs[tt][:], start=True, stop=True)
                    w1t = scr.tile([128, 512], bf16, tag="ow", name="w1t")
                    nc.scalar.activation(out=w1t[:], in_=b_w[:],
                                         func=AF.Identity, bias=cd_s[tt][:])
                    s1t = scr.tile([128, 512], bf16, tag="ow", name="s1t")
                    nc.vector.tensor_tensor(out=s1t[:], in0=w1t[:],
                                            in1=q1s[tt][:], op=OP.subtract)
                    ot = scr.tile([128, 512], bf16, tag="ow", name="ot")
                    nc.vector.tensor_tensor(out=ot[:], in0=s1t[:],
                                            in1=q2s[tt][:], op=OP.subtract)
                    nc.sync.dma_start(
                        out=out[tt, :, t * NB : (t + 1) * NB], in_=ot[:]
                    )
                    yield

            tail_state = {"g": None}

            def tail_aux(n=1):
                g = tail_state["g"]
                if g is None:
                    return
                for _ in range(n):
                    try:
                        next(g)
                    except StopIteration:
                        tail_state["g"] = None
                        return

            def mk_gates(hyper):
                psg = tlt_()
                for k in range(4):
                    nc.tensor.matmul(
                        out=psg[0:8, :], lhsT=HW_s[:, k, :],
                        rhs=hyper[:, k, :],
                        start=(k == 0), stop=(k == 3),
                    )
                gates = gtp.tile([8, 512], bf16, tag="gates", name="gates")
                nc.scalar.activation(
                    out=gates[:], in_=psg[0:8, :], func=AF.Sigmoid,
                    bias=hb_s[:]
                )
                return gates

            # -------- tile 0: phased share towers first --------
            xT_cur, xTf_cur = mk_phase_tiles()
            sd_cur, sp_cur = make_selector(gs_cur, mq_c, xT_cur, xTf_cur)

            # tile-1 inputs + gathers up front so its selector exists for
            # tile-0's share-tower aux slots
            idx_n, mq_n, masks_n = load_tile_inputs(1)
            gs_next = issue_gathers(idx_n, 1)
            xT_next, xTf_next = mk_phase_tiles()
            sd_nxt, sp_nxt = make_selector(gs_next, mq_n, xT_next, xTf_next)

            def phase_cb(p):
                # emit the 4 dve+pe select units of phase p right before
                # the tower's first phase-p matmul
                sd_cur(4)
                sp_cur(4)

            # tile-1's round-0/1 select units ride the late aux slots of
            # tile-0's share towers (their gather data lands ~52-70us)
            S0_DVE = {19: 0, 21: 1, 23: 2, 25: 3, 28: 4, 30: 5, 32: 6,
                      34: 7}
            S0_PE = {22: 0, 24: 1, 26: 2, 28: 3, 31: 4, 33: 5, 35: 6,
                     37: 7}
            s0cnt = [0]

            def share0_aux(n=1):
                s0cnt[0] += 1
                if s0cnt[0] in S0_DVE:
                    sd_nxt(1)
                if s0cnt[0] in S0_PE:
                    sp_nxt(1)

            h0s0 = tower_l0(sW0k_s, sb0_s, xT_cur, share0_aux,
                            scale=1.0 / ESCALE, phase_cb=phase_cb)
            h1s0 = tower_l1(h0s0, sW1_s, sb1_s, share0_aux)

            # Selects are never interleaved into the DR gate towers (fp8
            # transposes inside DoubleRow accumulation groups proved flaky
            # on HW); tile-1's remaining selects are flushed after.
            h0g0 = tower_l0_dr(gW0_s, gb0_s, xTf_cur, noop)
            hyper0 = tower_l1_dr(h0g0, gW1_s, gb1_s, noop)
            gates_cur = mk_gates(hyper0)
            sd_nxt(NSUB)  # flush tile-1 selects before t=1's towers
            sp_nxt(NSUB)
            tail_state["g"] = tail_gen(0, h1s0, (lambda g=gates_cur: g),
                                       masks_c)
            masks_cur = masks_n
            xT_cur, xTf_cur = xT_next, xTf_next

            # -------- tiles 1..NT-1 --------
            for t in range(1, NT):
                masks_t = masks_cur
                if t + 1 < NT:
                    idx_n, mq_n, masks_cur = load_tile_inputs(t + 1)
                    gs_next = issue_gathers(idx_n, t + 1)
                    xT_next, xTf_next = mk_phase_tiles()
                    sel_d, sel_p = make_selector(gs_next, mq_n, xT_next,
                                                 xTf_next)
                else:
                    xT_next = xTf_next = None
                    sel_d = sel_p = noop

                xT, xTf = xT_cur, xTf_cur

                def gate_aux(n=1):
                    tail_aux(1)

                # 16 dve + 16 pe select units for tile t+1, spread across
                # the share towers' 40 aux slots (later units complete in
                # later gather rounds)
                SEL_DVE_PTS = {7: 0, 9: 1, 11: 2, 13: 3, 16: 4, 18: 5,
                               20: 6, 22: 7, 24: 8, 26: 9, 28: 10, 30: 11,
                               32: 12, 34: 13, 36: 14, 38: 15}
                SEL_PE_PTS = {10: 0, 12: 1, 14: 2, 17: 3, 19: 4, 21: 5,
                              23: 6, 25: 7, 27: 8, 29: 9, 31: 10, 33: 11,
                              35: 12, 37: 13, 39: 14, 40: 15}
                share_cnt = [0]

                def share_aux(n=1):
                    share_cnt[0] += 1
                    if share_cnt[0] in SEL_DVE_PTS:
                        sel_d(1)
                    if share_cnt[0] in SEL_PE_PTS:
                        sel_p(1)
                    tail_aux(1)

                if t < NT - 1:
                    # gate towers drive the previous tile's tail; share
                    # towers drive the next tile's selects + leftover tail
                    h0g = tower_l0_dr(gW0_s, gb0_s, xTf, gate_aux)
                    hyper = tower_l1_dr(h0g, gW1_s, gb1_s, gate_aux)
                    gates = mk_gates(hyper)
                    h0s = tower_l0(sW0k_s, sb0_s, xT, share_aux,
                                   scale=1.0 / ESCALE)
                    h1s = tower_l1(h0s, sW1_s, sb1_s, share_aux)
                    tail_aux(100)  # finish previous tile's tail
                    sel_d(NSUB)  # flush remaining selects
                    sel_p(NSUB)
                    tail_state["g"] = tail_gen(t, h1s, (lambda g=gates: g),
                                               masks_t)
                else:
                    # last tile: share towers first (driving the previous
                    # tail), then its own tail interleaves into the gate
                    # towers; epilogue chunks (needing gates) come after
                    h0s = tower_l0(sW0k_s, sb0_s, xT, share_aux,
                                   scale=1.0 / ESCALE)
                    h1s = tower_l1(h0s, sW1_s, sb1_s, share_aux)
                    tail_aux(100)
                    cell = [None]
                    tail_state["g"] = tail_gen(t, h1s, (lambda: cell[0]),
                                               masks_t, last=True)
                    cap = [0]

                    def gate_aux_last(n=1):
                        cap[0] += 1
                        if cap[0] <= 20:
                            tail_aux(1)
                        if cap[0] in (5, 8, 11, 14, 17, 20, 21):
                            tail_aux(1)  # 27 pre-gate chunks total

                    h0g = tower_l0_dr(gW0_s, gb0_s, xTf, gate_aux_last)
                    hyper = tower_l1_dr(h0g, gW1_s, gb1_s, gate_aux_last)
                    cell[0] = mk_gates(hyper)
                    tail_aux(100)
                xT_cur, xTf_cur = xT_next, xTf_next

    nc.compile()
    _BUILT["nc"] = nc
    return nc


def _prep_core(x_c, d_c):
    """Build idx16 (128, NT*NSUB*NIDX/16) int16 and quarter masks
    (128, NT*3*128) uint8."""
    xv = np.asarray(x_c, dtype=np.int64)  # (BC, F)
    idx16 = np.zeros((16, NT * NSUB * NIDX // 16), dtype=np.int16)
    mq = np.zeros((P, NT * 3 * P), dtype=np.uint8)
    for t in range(NT):
        xt = xv[t * NB : (t + 1) * NB]  # (512, 32)
        vj = np.empty((P, P), dtype=np.int64)  # [j, p], j = cc*32 + f
        for cc in range(4):
            blk = xt[cc * 128 : (cc + 1) * 128]  # (128 p, 32 f)
            vj[cc * 32 : (cc + 1) * 32, :] = blk.T
        qv = (vj & 3).T  # [p, j]
        for q in (1, 2, 3):
            mq[:, (t * 3 + q - 1) * P : (t * 3 + q) * P] = (qv == q)
        hi = (vj >> 2).astype(np.int16)  # [j, p]
        # issue-order layout: chunk p holds round p's 4 sub-gathers
        for p in range(4):
            for cc in range(4):
                s4 = cc * 4 + p
                sub = hi[s4 * SLOT : (s4 + 1) * SLOT, :]  # [c, p]; i=c*128+p
                flat = sub.reshape(-1)
                wrapped = flat.reshape(NIDX // 16, 16).T
                c0 = t * (NSUB * NIDX // 16) + (p * 4 + cc) * (NIDX // 16)
                idx16[:, c0 : c0 + NIDX // 16] = wrapped
    idx_full = np.tile(idx16, (8, 1))
    dv = np.asarray(d_c, dtype=np.int64)
    import ml_dtypes
    bf = ml_dtypes.bfloat16
    dmb = (dv[None, :] == (np.arange(P) // 16)[:, None]).astype(bf)
    dm8 = (dv[None, :] == np.arange(8)[:, None]).astype(bf)
    return idx_full, mq, dmb, dm8


TRACE = False
LAST_RESULT = None


def kernel(**inputs):
    import ml_dtypes
    from concourse.bass_utils import run_bass_kernel_spmd

    nc = build()

    f32 = np.float32
    bf = ml_dtypes.bfloat16
    f8 = ml_dtypes.float8_e3m4
    e4 = ml_dtypes.float8_e4m3fn
    emb = np.asarray(inputs["emb"], dtype=f32)
    emb4 = np.ascontiguousarray((emb.reshape(V4, E4) * ESCALE).astype(f8))
    x = np.asarray(inputs["x"], dtype=np.int64)
    d = np.asarray(inputs["d"], dtype=np.int64)

    Tb0 = np.asarray(inputs["Tb0"], f32)
    Tlb0 = np.asarray(inputs["Tlb0"], f32)
    Tb1 = np.asarray(inputs["Tb1"], f32)
    Tlb1 = np.asarray(inputs["Tlb1"], f32)
    Tk1 = np.asarray(inputs["Tk1"], f32)
    TA1 = np.asarray(inputs["TA1"], f32)
    TB1 = np.asarray(inputs["TB1"], f32)
    TB0 = np.asarray(inputs["TB0"], f32)
    TA0 = np.asarray(inputs["TA0"], f32)
    DA0 = np.asarray(inputs["DA0"], f32)
    DB0 = np.asarray(inputs["DB0"], f32)
    Dlb0 = np.asarray(inputs["Dlb0"], f32)
    h0W = np.asarray(inputs["h0W"], f32)
    h1W = np.asarray(inputs["h1W"], f32)
    h0b = np.asarray(inputs["h0b"], f32)
    h1b = np.asarray(inputs["h1b"], f32)

    pk1 = []
    for tt in range(2):
        pkt = np.zeros((128, 32), f32)
        pkt[:, 0:16] = TA1[tt]
        pkt[:, 16] = Tk1[:, 0]
        pk1.append(pkt)

    # rank-1 broadcast matrices
    FD1 = []
    LB1 = []
    for tt in range(2):
        fd = np.zeros((32, 128), f32)
        fd[0:16, :] = TB1[tt]  # (16,1) broadcast along cols
        fd[16, :] = 1.0
        FD1.append(fd)
        LB1.append(np.repeat(TB1[tt], 128, axis=1))
    K1b = np.repeat(Tk1, 128, axis=1)

    def c128(v):
        return np.full((128, 1), v, f32)

    def kmajor(W, k, dtype=f32):
        """Rearrange (k*128, N) so a row-major DMA into a (128, k, N) SBUF
        tile yields tile[p, j] = W[j*128 + p]."""
        W = np.asarray(W, f32)
        n = W.shape[1] if W.ndim > 1 else 1
        return np.ascontiguousarray(
            W.reshape(k, 128, n).transpose(1, 0, 2).reshape(k * 128, n)
        ).astype(dtype)

    G8sel = np.zeros((8, 8 * 128), f32)
    for r in range(8):
        G8sel[r, r * 128 : (r + 1) * 128] = 1.0

    sW0r = np.asarray(inputs["sW0"], f32).reshape(16, 128, 1024)

    shared = {
        "emb4": emb4,
        "gW0": np.ascontiguousarray(
            np.asarray(inputs["gW0"], f32).reshape(8, 2, 128, 1024)
            .transpose(2, 0, 1, 3).reshape(2048, 1024)).astype(e4),
        "sW1": kmajor(inputs["sW1"], 8, bf),
        "gW1": np.ascontiguousarray(
            np.asarray(inputs["gW1"], f32).reshape(4, 2, 128, 512)
            .transpose(2, 0, 1, 3).reshape(1024, 512)).astype(e4),
        "sb0": kmajor(np.asarray(inputs["sb0"], f32).reshape(1024, 1), 8),
        "gb0": kmajor(np.asarray(inputs["gb0"], f32).reshape(1024, 1), 8),
        "sb1": kmajor(np.asarray(inputs["sb1"], f32).reshape(512, 1), 4),
        "gb1": kmajor(np.asarray(inputs["gb1"], f32).reshape(512, 1), 4),
        "Dk0": kmajor(inputs["Dk0"], 4, bf),
        "Db0": kmajor(np.asarray(inputs["Db0"], f32).reshape(256, 1), 2),
        "DA0p": kmajor(DA0.transpose(1, 0, 2).reshape(512, 128), 4, bf),
        "DB0p": np.ascontiguousarray(DB0.reshape(128, 256)).astype(bf),
        "Dlb0p": np.ascontiguousarray(Dlb0).astype(bf),
        "Tk0": kmajor(inputs["Tk0"], 2, bf),
        "TA0p": kmajor(np.concatenate([TA0[0], TA0[1]], axis=1), 2, bf),
        "TB0p_0": np.concatenate([TB0[0], np.zeros((16, 128), f32)]).astype(bf),
        "TB0p_1": np.concatenate([np.zeros((16, 128), f32), TB0[1]]).astype(bf),
        "bias_d0": (Tb0 + Tlb0[0]).reshape(128, 1),
        "bias_d1": (Tb0 + Tlb0[1]).reshape(128, 1),
        "bias_s": Tb0.reshape(128, 1),
        "bias_l0": Tlb0[0].reshape(128, 1),
        "bias_l1": Tlb0[1].reshape(128, 1),
        "pk1_0": pk1[0].astype(bf),
        "pk1_1": pk1[1].astype(bf),
        "TA1_0": np.ascontiguousarray(TA1[0]).astype(bf),
        "TA1_1": np.ascontiguousarray(TA1[1]).astype(bf),
        "FD1_0": FD1[0].astype(bf),
        "FD1_1": FD1[1].astype(bf),
        "LB1_0": LB1[0].astype(bf),
        "LB1_1": LB1[1].astype(bf),
        "K1b": K1b.astype(bf),
        "c_d0": c128(Tb1[0] + Tlb1[0, 0]),
        "c_d1": c128(Tb1[0] + Tlb1[1, 0]),
        "c_s": c128(Tb1[0]),
        "c_l0": c128(Tlb1[0, 0]),
        "c_l1": c128(Tlb1[1, 0]),
        "HWp": kmajor(np.concatenate([h0W, h1W], axis=1), 4, bf),
        "hb": np.concatenate([h0b, h1b]).reshape(8, 1),
        "ident8": np.eye(128, dtype=f32).astype(f8),
        "G8sel": G8sel.astype(bf),
    }
    for g in range(NPH):
        shared[f"sW0k{g}"] = np.ascontiguousarray(
            sW0r[4 * g : 4 * g + 4].transpose(1, 0, 2).reshape(512, 1024)
        ).astype(bf)

    in_maps = []
    for c in range(NCORES):
        idx_full, mq, dmb, dm8 = _prep_core(
            x[c * BC : (c + 1) * BC], d[c * BC : (c + 1) * BC]
        )
        m = dict(shared)
        m["idxd"] = idx_full
        m["mqd"] = mq
        m["dmb"] = dmb
        m["dm8"] = dm8
        in_maps.append(m)

    res = run_bass_kernel_spmd(nc, in_maps, list(range(NCORES)), trace=TRACE)
    global LAST_RESULT
    LAST_RESULT = res
    outs = [np.asarray(res.results[c]["out"]).transpose(0, 2, 1)
            for c in range(NCORES)]
    return np.concatenate(outs, axis=1).astype(np.float32)  # (2, B, 128)
